# revision 54
# baseline (speedup 1.0000x reference)
"""DualAttention (cross+self bidirectional attention, 2 streams) on 8 TRN2 cores.

Sharding: data-parallel over batch (4) x tensor-parallel over heads (2 groups
of 8 heads). Core c handles batch c//2, head-group c%2. Each core computes its
head-group's slice of all 6 input projections, RoPE, the 4 attention combos,
and a partial output projection; the host sums the two partial out-projections
per batch and adds the output bias.

Device-side structure (v2):
  - q/k projections computed TRANSPOSED ([e_out, s]) in head-contiguous order
    (slab s = heads 2s,2s+1, each head's 64 dims contiguous), so attention
    scores need ONE 64-deep matmul per (head, key-chunk) instead of two
    32-deep ones. RoPE's rotate_half is a fixed 128x128 permutation matmul
    (zero rows on pass-through dims) plus elementwise combines with cos rows
    set to 1 on pass-through dims.
  - keys are host-packed (masked keys dropped, padded to SKP): masking becomes
    a -30000 per-partition exp bias on pad rows and attention shrinks ~2x.
  - V is augmented with a ones column per head (stride-65 layout): the AV
    matmul emits both O (rows 0-63) and the softmax denominator r (row 64)
    in one pass, so no separate row-sum matmuls are needed.
  - normalization: 1/r per head via DVE reciprocal into one row, then gpsimd
    partition_broadcast replicates it across partitions; O is normalized by
    DVE multiplies while accumulating the two combos per output. Head B's O
    needs a partition shift (psum rows 0-63 -> att rows 64-127) done by DMA.
  - matmuls in float32r (tf32-class, full PE rate); U/V/att/Wo in bf16.
  - out-projection writes PSUM->DRAM directly via DMA.
  - emission is generator-woven so projection/attention/out-projection work
    interleaves in every engine's instruction stream (phases overlap).
"""

import numpy as np

B, S, E, H = 4, 1024, 1024, 16
D, R = 64, 32
HG, EG = 8, 512  # heads / e-columns per head-group
P = 128
KE = E // P  # contraction chunks of a projection
SCALE = D ** -0.5
NCORES = 8
MASK_BIAS = -30000.0
# Augmented-V layout: per head pair, even head gets [V(64)|ones] (65 cols,
# AV out rows 0-63 = O, row 64 = r); odd head gets [zeros(63)|ones|V(64)]
# (128 cols, AV out row 63 = r, rows 64-127 = O). Zero columns are free on
# the PE (matmul cost is moving-row count), and both heads' O land at the
# att-tile partition offsets directly -- no partition-shift DMA needed.
VPAIR = 193  # 65 + 128
USE_SELMAT = False  # fallback: replicate 1/r via PE matmul, not gpsimd bcast
import os as _os
ATT_SPLIT = _os.environ.get("K_ATT_SPLIT", "1") == "1"
INIT_WEAVE = _os.environ.get("K_INIT_WEAVE", "1") == "1"
XP_HW = _os.environ.get("K_XP_HW", "1") == "1"

_PROG_CACHE = {}


def _slices(SF):
    """512-wide slices, except a trailing 640 remainder splits 384+256 so
    every f32r matmul free dim is >= 256 (below 256 f32r runs at 1/4 rate)."""
    out = []
    lo = 0
    while lo < SF:
        rem = SF - lo
        if rem > 512:
            step = 512 if rem >= 768 else 384
        else:
            step = rem
        out.append((lo, lo + step))
        lo += step
    return out


def _build_program(SKP, reps=1, parts="pako"):
    key = (SKP, reps, parts)
    if key in _PROG_CACHE:
        return _PROG_CACHE[key]

    import concourse.bass as bass
    import concourse.tile as tile
    from concourse import bacc, mybir
    from contextlib import ExitStack

    f32 = mybir.dt.float32
    f32r = mybir.dt.float32r
    bf16 = mybir.dt.bfloat16
    NKC = SKP // P
    ts = bass.ts

    nc = bacc.Bacc("TRN2", target_bir_lowering=False, debug=False, num_devices=NCORES)

    def din(name, shape, dt):
        return nc.dram_tensor(name, list(shape), dt, kind="ExternalInput").ap()

    a = {}
    a["x1T"] = din("x1T", (E, S), bf16)
    a["x2T"] = din("x2T", (E, S), bf16)
    a["x1p"] = din("x1p", (E, SKP), bf16)
    a["x2p"] = din("x2p", (E, SKP), bf16)
    for n in ("q1", "q2", "k1", "k2", "v1", "v2"):
        a["W" + n] = din("W" + n, (E, EG), bf16)
    a["Wo1"] = din("Wo1", (EG, S), bf16)
    a["Wo2"] = din("Wo2", (EG, S), bf16)
    for n in ("q1", "q2", "k1", "k2"):
        a["b" + n] = din("b" + n, (P, 4), f32)
    a["bv1"] = din("bv1", (P, EG), f32)
    a["bv2"] = din("bv2", (P, EG), f32)
    for n in ("cq1", "sq1", "cq2", "sq2"):
        a[n] = din(n, (P, S), bf16)
    for n in ("ck1", "sk1", "ck2", "sk2"):
        a[n] = din(n, (P, SKP), bf16)
    a["mb1"] = din("mb1", (P, NKC), f32)
    a["mb2"] = din("mb2", (P, NKC), f32)
    a["perm"] = din("perm", (P, P), bf16)
    if USE_SELMAT:
        a["selmat"] = din("selmat", (P, P), f32)
        a["zeros"] = din("zeros", (P, 1024), f32)
    out1 = nc.dram_tensor("o1", [S, E], bf16, kind="ExternalOutput").ap()
    out2 = nc.dram_tensor("o2", [S, E], bf16, kind="ExternalOutput").ap()

    Exp = mybir.ActivationFunctionType.Exp

    def emit(tc):
        with ExitStack() as ctx:
            consts = ctx.enter_context(tc.tile_pool(name="consts", bufs=1))
            xpool = ctx.enter_context(tc.tile_pool(name="xpool", bufs=1))
            xpp = ctx.enter_context(tc.tile_pool(name="xpp", bufs=1))
            wpool = ctx.enter_context(tc.tile_pool(name="wpool", bufs=6))
            qkv = ctx.enter_context(tc.tile_pool(name="qkv", bufs=1))
            attp = ctx.enter_context(tc.tile_pool(name="attp", bufs=1))
            upool = ctx.enter_context(tc.tile_pool(name="upool", bufs=2))
            sm = ctx.enter_context(tc.tile_pool(name="sm", bufs=2))
            # PSUM: ST 2x[P,1024](4 banks) + proj 2x[P,512](2) + o 2x[P,512](2)
            stps = ctx.enter_context(tc.tile_pool(name="stps", bufs=2, space="PSUM"))
            pjps = ctx.enter_context(tc.tile_pool(name="pjps", bufs=2, space="PSUM"))
            ops_ = ctx.enter_context(tc.tile_pool(name="ops", bufs=2, space="PSUM"))

            dmas = [nc.sync, nc.scalar]
            dma_ctr = [0]

            def next_dma():
                e = dmas[dma_ctr[0] % 2]
                dma_ctr[0] += 1
                return e

            def cload(name, shape, dt):
                t = consts.tile(list(shape), dt, tag=name)
                nc.gpsimd.dma_start(t[:], a[name])
                return t

            perm_t = cload("perm", (P, P), bf16)
            mb = {1: cload("mb1", (P, NKC), f32), 2: cload("mb2", (P, NKC), f32)}
            cq = {n: cload(n, (P, S), bf16) for n in ("cq1", "sq1", "cq2", "sq2")}
            ck = {n: cload(n, (P, SKP), bf16) for n in ("ck1", "sk1", "ck2", "sk2")}
            bqk = {n: cload("b" + n, (P, 4), f32) for n in ("q1", "q2", "k1", "k2")}
            bv = {n: cload("b" + n, (P, EG), f32) for n in ("v1", "v2")}
            if USE_SELMAT:
                selmat = cload("selmat", (P, P), f32)
                zrc = [consts.tile([P, 1024], f32, tag=f"zrc{i}", name=f"zrc{i}")
                       for i in range(2)]
                for i in range(2):
                    nc.gpsimd.dma_start(zrc[i][:], a["zeros"])

            def body(_=None):
                # ---------- generators (work-unit granularity) ----------
                def load_x(pool, slot_tag, ap, SF, split):
                    t = pool.tile([P, KE, SF], bf16, tag=slot_tag, name="xt")
                    src = ap.rearrange("(ko p) s -> p ko s", p=P)
                    if XP_HW:
                        next_dma().dma_start(t[:, :, 0:split], src[:, :, 0:split])
                        next_dma().dma_start(t[:, :, split:SF], src[:, :, split:SF])
                    else:
                        nc.gpsimd.dma_start(t[:], src)
                    return t

                def load_w_halves(wname, dt_, SFW):
                    ko = a[wname].shape[0] // P // 2  # k-chunks per half
                    tiles = []
                    for h in range(2):
                        w_t = wpool.tile([P, ko, SFW], dt_, tag="w", name="wt")
                        next_dma().dma_start(
                            w_t[:],
                            a[wname].rearrange("(h ko p) m -> h p ko m", h=2, p=P)[h],
                        )
                        tiles.append(w_t)
                    return tiles

                def proj_qk(dst, x_t, wname, SF, cos_t, sin_t, w_ts=None):
                    """Generator: dst [P, 4, SF] bf16; yields per (mi, nb)."""
                    if w_ts is None:
                        w_ts = load_w_halves("W" + wname, bf16, EG)
                    slices = _slices(SF)
                    for mi in range(4):
                        for lo, hi in slices:
                            ps = pjps.tile([P, 512], f32, tag="pj", name="pj")
                            for ki in range(KE):
                                nc.tensor.matmul(
                                    ps[:, : hi - lo],
                                    w_ts[ki // 4][:, ki % 4, ts(mi, P)],
                                    x_t[:, ki, lo:hi],
                                    start=(ki == 0),
                                    stop=(ki == KE - 1),
                                )
                            nc.vector.tensor_scalar_add(
                                dst[:, mi, lo:hi],
                                ps[:, : hi - lo],
                                bqk[wname][:, mi : mi + 1],
                            )
                            # rope (every slab has rot rows 0-31 / 64-95)
                            pp = pjps.tile([P, 512], f32, tag="pj", name="pp")
                            nc.tensor.matmul(
                                pp[:, : hi - lo], perm_t[:], dst[:, mi, lo:hi],
                                start=True, stop=True,
                            )
                            tmp = sm.tile([P, 512], bf16, tag="tmp", name="tmp")
                            nc.vector.tensor_mul(
                                tmp[:, : hi - lo], pp[:, : hi - lo],
                                sin_t[:, lo:hi],
                            )
                            nc.vector.tensor_mul(
                                dst[:, mi, lo:hi], dst[:, mi, lo:hi],
                                cos_t[:, lo:hi],
                            )
                            nc.vector.tensor_add(
                                dst[:, mi, lo:hi], dst[:, mi, lo:hi],
                                tmp[:, : hi - lo],
                            )
                            yield

                def proj_v(dst, xp_t, wname):
                    """Generator: dst [P, NKC, 4*VPAIR] bf16; yields per mi."""
                    w_ts = load_w_halves("W" + wname, bf16, EG)
                    dstv = dst[:].rearrange("p k (pr c) -> p k pr c", c=VPAIR)
                    psv = lambda t: t.rearrange("p (pr h c) -> p pr h c", pr=4, h=2)
                    bvv = psv(bv[wname][:])
                    for mi in range(NKC):
                        ps = pjps.tile([P, 512], f32, tag="pj", name="pjv")
                        for ki in range(KE):
                            nc.tensor.matmul(
                                ps[:],
                                xp_t[:, ki, ts(mi, P)],
                                w_ts[ki // 4][:, ki % 4, :],
                                start=(ki == 0),
                                stop=(ki == KE - 1),
                            )
                        pp = psv(ps[:])
                        nc.vector.tensor_add(
                            dstv[:, mi, :, 0:D], pp[:, :, 0, :], bvv[:, :, 0, :]
                        )
                        nc.vector.tensor_add(
                            dstv[:, mi, :, 129 : 129 + D],
                            pp[:, :, 1, :], bvv[:, :, 1, :],
                        )
                        yield

                def attention(att_t, first, qT, kT, v_t, mbias):
                    """Generator: yields per (sq, gp, ki)x2 and per normalize.
                    All NKC score/exp units are emitted before the AV chain so
                    the o-psum slot wait (previous pair's normalize) hides
                    under score work on the in-order PE stream."""
                    do_k = "k" in parts
                    ut_bufs = (NKC + 1) if ATT_SPLIT else 2
                    for sq in range(2):
                        for gp in range(4):  # slab = heads (2gp, 2gp+1)
                            hA, hB = 2 * gp, 2 * gp + 1
                            uts = []
                            if not ATT_SPLIT and do_k:
                                oA = ops_.tile([P, 512], f32, tag="o", name="oA")
                                oB = ops_.tile([P, 512], f32, tag="o", name="oB")
                            for ki in range(NKC):
                                st = stps.tile([P, 1024], f32, tag="st", name="st")
                                for half, par in ((0, 0), (1, 1)):
                                    nc.tensor.matmul(
                                        st[:, 512 * par : 512 * par + 512],
                                        kT[64 * half : 64 * half + 64, gp, ts(ki, P)],
                                        qT[64 * half : 64 * half + 64, gp, ts(sq, 512)],
                                        start=True,
                                        stop=True,
                                    )
                                ut = upool.tile([P, 1024], bf16, tag="ut",
                                                name="ut", bufs=ut_bufs)
                                nc.scalar.activation(
                                    ut[:], st[:], Exp,
                                    bias=mbias[:, ki : ki + 1], scale=SCALE,
                                )
                                uts.append(ut)
                                if not ATT_SPLIT and do_k:
                                    for ki2 in (ki,):
                                        nc.tensor.matmul(
                                            oA[0:65, :],
                                            v_t[:, ki2, VPAIR * gp : VPAIR * gp + 65],
                                            ut[:, 0:512],
                                            start=(ki2 == 0),
                                            stop=(ki2 == NKC - 1),
                                        )
                                        nc.tensor.matmul(
                                            oB[:, :],
                                            v_t[:, ki2, VPAIR * gp + 65 : VPAIR * gp + 193],
                                            ut[:, 512:1024],
                                            start=(ki2 == 0),
                                            stop=(ki2 == NKC - 1),
                                        )
                                yield
                            if ATT_SPLIT and do_k:
                                oA = ops_.tile([P, 512], f32, tag="o", name="oA")
                                oB = ops_.tile([P, 512], f32, tag="o", name="oB")
                                for ki in range(NKC):
                                    nc.tensor.matmul(
                                        oA[0:65, :],
                                        v_t[:, ki, VPAIR * gp : VPAIR * gp + 65],
                                        uts[ki][:, 0:512],
                                        start=(ki == 0),
                                        stop=(ki == NKC - 1),
                                    )
                                    nc.tensor.matmul(
                                        oB[:, :],
                                        v_t[:, ki, VPAIR * gp + 65 : VPAIR * gp + 193],
                                        uts[ki][:, 512:1024],
                                        start=(ki == 0),
                                        stop=(ki == NKC - 1),
                                    )
                                    yield
                            if not do_k:
                                continue
                            # normalize: full-tile 1/r (single-row custom-DVE
                            # recip is broken); r_A sits at oA row 64, r_B at
                            # oB row 63. One DMA stages recip rows 63-64 to
                            # partitions 0-1, partition_broadcast replicates,
                            # then partition-aligned multiplies.
                            rc = sm.tile([P, 1024], f32, tag="rc", name="rc")
                            nc.vector.reciprocal_approx_fast(
                                rc[:, 0:512], oA[:]
                            )
                            nc.vector.reciprocal_approx_fast(
                                rc[:, 512:1024], oB[:]
                            )
                            stage = sm.tile([P, 1024], f32, tag="stg",
                                            name="stg")
                            nc.gpsimd.dma_start(
                                stage[0:1, 0:512], rc[64:65, 0:512]
                            )
                            nc.gpsimd.dma_start(
                                stage[0:1, 512:1024], rc[63:64, 512:1024]
                            )
                            # pbcast writes must start at partition 0; head B
                            # needs its factor at partitions 64-127, so its
                            # broadcast covers all 128 (separate columns)
                            rrs = sm.tile([P, 1024], f32, tag="rrs",
                                          name="rrs")
                            nc.gpsimd.partition_broadcast(
                                rrs[0:64, 0:512], stage[0:1, 0:512]
                            )
                            nc.gpsimd.partition_broadcast(
                                rrs[0:128, 512:1024], stage[0:1, 512:1024]
                            )
                            dst = att_t[:, gp, ts(sq, 512)]
                            if first:
                                nc.vector.tensor_mul(
                                    att_t[0:64, gp, ts(sq, 512)],
                                    oA[0:64, :], rrs[0:64, 0:512],
                                )
                                nc.vector.tensor_mul(
                                    att_t[64:128, gp, ts(sq, 512)],
                                    oB[64:128, :], rrs[64:128, 512:1024],
                                )
                            else:
                                tmpo = sm.tile([P, 512], bf16, tag="tmpo", name="tm")
                                nc.vector.tensor_mul(
                                    tmpo[0:64, :], oA[0:64, :], rrs[0:64, 0:512]
                                )
                                nc.vector.tensor_mul(
                                    tmpo[64:128, :], oB[64:128, :],
                                    rrs[64:128, 512:1024],
                                )
                                nc.vector.tensor_add(dst, dst, tmpo[:])
                            yield

                def outproj(att_t, wo_name, out_ap, wo_ts=None):
                    """Generator: yields per si."""
                    if wo_ts is None:
                        wo_ts = load_w_halves(wo_name, bf16, S)
                    for si in range(8):
                        osb = sm.tile([P, 1024], bf16, tag="ob", name="ob")
                        for fb in range(2):
                            ps = pjps.tile([P, 512], f32, tag="pj", name="pjo")
                            for ei in range(4):
                                nc.tensor.matmul(
                                    ps[:],
                                    att_t[:, ei, ts(si, P)],
                                    wo_ts[ei // 2][:, ei % 2, 512 * fb : 512 * fb + 512],
                                    start=(ei == 0),
                                    stop=(ei == 3),
                                )
                            nc.vector.tensor_copy(
                                osb[:, 512 * fb : 512 * fb + 512], ps[:]
                            )
                        next_dma().dma_start(out_ap[ts(si, P), :], osb[:])
                        yield

                def drain(*gens):
                    for g in gens:
                        for _ in g:
                            pass

                def drain_n(g, n):
                    for _ in range(n):
                        try:
                            next(g)
                        except StopIteration:
                            return False
                    return True

                def weave(mains, sides=(), ratio=3):
                    """Pull `ratio` units per round from each main gen, then
                    ONE unit from the head of the side chain (sides drain
                    sequentially -- this keeps per-pool slot demand bounded).
                    sides: [(gen, cap), ...]; cap limits units pulled here."""
                    mains = [[g, False] for g in mains]
                    chain = [[g, cap] for g, cap in sides]
                    while True:
                        alive = False
                        for rec in mains:
                            if rec[1]:
                                continue
                            for _ in range(ratio):
                                try:
                                    next(rec[0])
                                    alive = True
                                except StopIteration:
                                    rec[1] = True
                                    break
                        while chain:
                            g, cap = chain[0]
                            if cap <= 0:
                                chain.pop(0)
                                continue
                            try:
                                next(g)
                                chain[0][1] -= 1
                            except StopIteration:
                                chain.pop(0)
                                continue
                            break
                        if not alive:
                            break

                # ---------- tiles ----------
                qT1 = qkv.tile([P, 4, S], bf16, tag="qT1")
                qT2 = qkv.tile([P, 4, S], bf16, tag="qT2")
                kT1 = qkv.tile([P, 4, SKP], bf16, tag="kT1")
                kT2 = qkv.tile([P, 4, SKP], bf16, tag="kT2")
                v1 = qkv.tile([P, NKC, 4 * VPAIR], bf16, tag="v1")
                v2 = qkv.tile([P, NKC, 4 * VPAIR], bf16, tag="v2")
                att1 = attp.tile([P, 4, S], bf16, tag="att1")
                att2 = attp.tile([P, 4, S], bf16, tag="att2")

                # ones / zero columns of augmented V (proj_v writes V slots)
                for v_t in (v1, v2):
                    vv = v_t[:].rearrange("p k (pr c) -> p k pr c", c=VPAIR)
                    nc.vector.memset(vv[:, :, :, D : D + 1], 1.0)
                    nc.vector.memset(vv[:, :, :, 128:129], 1.0)
                    nc.vector.memset(vv[:, :, :, 65:128], 0.0)

                # ---------- schedule ----------
                wq1_ts = load_w_halves("Wq1", bf16, EG)
                x1t = load_x(xpool, "x", a["x1T"], S, 512)
                xp2 = load_x(xpp, "xp", a["x2p"], SKP, 384)
                g_q1 = proj_qk(qT1, x1t, "q1", S, cq["cq1"], cq["sq1"],
                               w_ts=wq1_ts)
                g_k2 = proj_qk(kT2, xp2, "k2", SKP, ck["ck2"], ck["sk2"])
                g_v2 = proj_v(v2, xp2, "v2")
                if INIT_WEAVE:
                    weave([g_q1], sides=[(g_k2, 99), (g_v2, 99)], ratio=1)
                drain(g_q1, g_k2, g_v2)

                do_a = "a" in parts
                xp1 = load_x(xpp, "xp", a["x1p"], SKP, 384)
                x2t = load_x(xpool, "x", a["x2T"], S, 512)
                g_k1 = proj_qk(kT1, xp1, "k1", SKP, ck["ck1"], ck["sk1"])
                g_v1 = proj_v(v1, xp1, "v1")
                g_q2 = proj_qk(qT2, x2t, "q2", S, cq["cq2"], cq["sq2"])
                if do_a:
                    # combo unit counts per sq-half
                    upg = (2 * NKC + 1) if ATT_SPLIT else (NKC + 1)
                    half_units = 4 * upg if "k" in parts else 4 * NKC
                    c0 = attention(att1, True, qT1, kT2, v2, mb[2])
                    weave([c0], sides=[(g_k1, 99), (g_v1, 99), (g_q2, 99)], ratio=3)
                    drain(g_k1, g_v1, g_q2)
                    wo1_ts = load_w_halves("Wo1", bf16, S)
                    wo2_ts = load_w_halves("Wo2", bf16, S)
                    c1 = attention(att1, False, qT1, kT1, v1, mb[1])
                    c2 = attention(att2, True, qT2, kT1, v1, mb[1])
                    c3 = attention(att2, False, qT2, kT2, v2, mb[2])
                    if "o" in parts:
                        op1 = outproj(att1, "Wo1", out1, wo_ts=wo1_ts)
                        op2 = outproj(att2, "Wo2", out2, wo_ts=wo2_ts)
                        # op si<4 needs sq0 of its att complete; si>=4 needs all
                        drain_n(c1, half_units)
                        weave([c1], sides=[(op1, 4)], ratio=3)
                        weave([c2], sides=[(op1, 99)], ratio=3)
                        drain(op1)
                        drain_n(c3, half_units)
                        weave([c3], sides=[(op2, 4)], ratio=3)
                        drain(op2)
                    else:
                        drain(c1, c2, c3)
                        z = sm.tile([P, 1024], f32, tag="ob")
                        nc.vector.memset(z[:], 0.0)
                        nc.sync.dma_start(out1[0:P, :], z[:])
                        nc.sync.dma_start(out2[0:P, :], z[:])
                else:
                    drain(g_k1, g_v1, g_q2)
                    z = sm.tile([P, 1024], f32, tag="ob")
                    nc.vector.memset(z[:], 0.0)
                    nc.sync.dma_start(out1[0:P, :], z[:])
                    nc.sync.dma_start(out2[0:P, :], z[:])

            if reps > 1:
                with tc.For_i(
                    0, reps, 1,
                    hint_engines=(
                        mybir.EngineType.PE,
                        mybir.EngineType.Activation,
                        mybir.EngineType.DVE,
                        mybir.EngineType.SP,
                        mybir.EngineType.Pool,
                    ),
                ):
                    body()
            else:
                body()

    with tile.TileContext(nc) as tc:
        emit(tc)
    nc.compile()
    _PROG_CACHE[key] = nc
    return nc


def _prep_inputs(inputs):
    """Host-side sharding/packing. Returns (in_maps, SKP, bo1, bo2)."""
    f32 = np.float32
    x1 = np.asarray(inputs["x1"], f32)
    x2 = np.asarray(inputs["x2"], f32)
    m1 = np.asarray(inputs["x1_padding_mask"]).astype(np.int64)
    m2 = np.asarray(inputs["x2_padding_mask"]).astype(np.int64)
    cos1 = np.asarray(inputs["cos1"], f32).reshape(S, R)
    sin1 = np.asarray(inputs["sin1"], f32).reshape(S, R)
    cos2 = np.asarray(inputs["cos2"], f32).reshape(S, R)
    sin2 = np.asarray(inputs["sin2"], f32).reshape(S, R)

    idx1 = [np.nonzero(m1[b])[0] for b in range(B)]
    idx2 = [np.nonzero(m2[b])[0] for b in range(B)]
    maxn = max([len(i) for i in idx1] + [len(i) for i in idx2] + [1])
    SKP = ((maxn + P - 1) // P) * P
    NKC = SKP // P

    import ml_dtypes

    bf16 = ml_dtypes.bfloat16

    gw = []
    for g in range(2):
        rows = slice(g * EG, (g + 1) * EG)
        d = {}
        for n in ("q1", "q2", "k1", "k2", "v1", "v2"):
            W = np.asarray(inputs["W" + n], f32)[rows]
            bb = np.asarray(inputs["b" + n], f32)[rows]
            d["W" + n] = np.ascontiguousarray(W.T).astype(bf16)
            if n[0] == "v":
                d["b" + n] = np.ascontiguousarray(
                    np.broadcast_to(bb[None, :], (P, EG))
                )
            else:
                d["b" + n] = np.ascontiguousarray(bb.reshape(4, P).T)
        for n in ("o1", "o2"):
            W = np.asarray(inputs["W" + n], f32)[:, rows]
            d["W" + n] = np.ascontiguousarray(W.T).astype(bf16)
        gw.append(d)

    def rope_rows(tab, fill):
        """[128, S-like] rows: per 64-block [32 rope rows | 32 fill rows],
        repeated for the 2 heads of a slab (all slabs identical)."""
        n = tab.shape[1]
        blk = np.concatenate(
            [tab, np.full((D - R, n), fill, f32)], axis=0
        )  # [64, n]
        return np.concatenate([blk, blk], axis=0)  # [128, n]

    cq = {
        "cq1": rope_rows(cos1.T, 1.0).astype(bf16),
        "sq1": rope_rows(sin1.T, 0.0).astype(bf16),
        "cq2": rope_rows(cos2.T, 1.0).astype(bf16),
        "sq2": rope_rows(sin2.T, 0.0).astype(bf16),
    }
    # rotate_half perm: rot rows at 0-31 / 64-95 of each slab; pass rows
    # (32-63 / 96-127) have all-zero columns.
    pm = np.zeros((P, P), f32)
    for o in (0, 64):
        for i in range(16):
            pm[o + 16 + i, o + i] = -1.0
            pm[o + i, o + 16 + i] = 1.0

    in_maps = []
    for c in range(NCORES):
        b, g = c // 2, c % 2
        m = {}
        m["x1T"] = np.ascontiguousarray(x1[b].T).astype(bf16)
        m["x2T"] = np.ascontiguousarray(x2[b].T).astype(bf16)
        for which, xb, idx, cos, sin in (
            (1, x1[b], idx1[b], cos1, sin1),
            (2, x2[b], idx2[b], cos2, sin2),
        ):
            n = len(idx)
            xp = np.zeros((SKP, E), f32)
            xp[:n] = xb[idx]
            m[f"x{which}p"] = np.ascontiguousarray(xp.T).astype(bf16)
            ckk = np.zeros((R, SKP), f32)
            skk = np.zeros((R, SKP), f32)
            ckk[:, :n] = cos.T[:, idx]
            skk[:, :n] = sin.T[:, idx]
            m[f"ck{which}"] = rope_rows(ckk, 1.0).astype(bf16)
            m[f"sk{which}"] = rope_rows(skk, 0.0).astype(bf16)
            mbv = np.full(SKP, MASK_BIAS, f32)
            mbv[:n] = 0.0
            m[f"mb{which}"] = np.ascontiguousarray(mbv.reshape(NKC, P).T)
        for n in ("q1", "q2", "k1", "k2", "v1", "v2"):
            m["W" + n] = gw[g]["W" + n]
            m["b" + n] = gw[g]["b" + n]
        m["Wo1"] = gw[g]["Wo1"]
        m["Wo2"] = gw[g]["Wo2"]
        m.update(cq)
        m["perm"] = pm.astype(bf16)
        if USE_SELMAT:
            sel = np.zeros((P, P), f32)
            sel[0, :64] = 1.0
            m["selmat"] = sel
            m["zeros"] = np.zeros((P, 1024), f32)
        in_maps.append(m)

    bo1 = np.asarray(inputs["bo1"], f32)
    bo2 = np.asarray(inputs["bo2"], f32)
    return in_maps, SKP, bo1, bo2


def kernel(**inputs):
    from concourse.bass_utils import run_bass_kernel_spmd

    in_maps, SKP, bo1, bo2 = _prep_inputs(inputs)
    nc = _build_program(SKP)
    res = run_bass_kernel_spmd(nc, in_maps, core_ids=list(range(NCORES)))
    f32 = np.float32
    o1 = np.stack(
        [res.results[2 * b]["o1"].astype(f32)
         + res.results[2 * b + 1]["o1"].astype(f32) + bo1 for b in range(B)]
    )
    o2 = np.stack(
        [res.results[2 * b]["o2"].astype(f32)
         + res.results[2 * b + 1]["o2"].astype(f32) + bo2 for b in range(B)]
    )
    return o1.astype(np.float32), o2.astype(np.float32)


# revision 55
# speedup vs baseline: 1.2756x; 1.2756x over previous
"""DualAttention (cross+self bidirectional attention, 2 streams) on 8 TRN2 cores.

Sharding: data-parallel over batch (4) x tensor-parallel over heads (2 groups
of 8 heads). Core c handles batch c//2, head-group c%2. Each core computes its
head-group's slice of all 6 input projections, RoPE, the 4 attention combos,
and a partial output projection; the host sums the two partial out-projections
per batch and adds the output bias.

Device-side structure (v2):
  - q/k projections computed TRANSPOSED ([e_out, s]) in head-contiguous order
    (slab s = heads 2s,2s+1, each head's 64 dims contiguous), so attention
    scores need ONE 64-deep matmul per (head, key-chunk) instead of two
    32-deep ones. RoPE's rotate_half is a fixed 128x128 permutation matmul
    (zero rows on pass-through dims) plus elementwise combines with cos rows
    set to 1 on pass-through dims.
  - keys are host-packed (masked keys dropped, padded to SKP): masking becomes
    a -30000 per-partition exp bias on pad rows and attention shrinks ~2x.
  - V is augmented with a ones column per head (stride-65 layout): the AV
    matmul emits both O (rows 0-63) and the softmax denominator r (row 64)
    in one pass, so no separate row-sum matmuls are needed.
  - normalization: 1/r per head via DVE reciprocal into one row, then gpsimd
    partition_broadcast replicates it across partitions; O is normalized by
    DVE multiplies while accumulating the two combos per output. Head B's O
    needs a partition shift (psum rows 0-63 -> att rows 64-127) done by DMA.
  - matmuls in float32r (tf32-class, full PE rate); U/V/att/Wo in bf16.
  - out-projection writes PSUM->DRAM directly via DMA.
  - emission is generator-woven so projection/attention/out-projection work
    interleaves in every engine's instruction stream (phases overlap).
"""

import numpy as np

B, S, E, H = 4, 1024, 1024, 16
D, R = 64, 32
HG, EG = 8, 512  # heads / e-columns per head-group
P = 128
KE = E // P  # contraction chunks of a projection
SCALE = D ** -0.5
NCORES = 8
MASK_BIAS = -30000.0
# Augmented-V layout: per head pair, even head gets [V(64)|ones] (65 cols,
# AV out rows 0-63 = O, row 64 = r); odd head gets [zeros(63)|ones|V(64)]
# (128 cols, AV out row 63 = r, rows 64-127 = O). Zero columns are free on
# the PE (matmul cost is moving-row count), and both heads' O land at the
# att-tile partition offsets directly -- no partition-shift DMA needed.
VPAIR = 193  # 65 + 128
USE_SELMAT = False  # fallback: replicate 1/r via PE matmul, not gpsimd bcast
import os as _os
ATT_SPLIT = _os.environ.get("K_ATT_SPLIT", "1") == "1"
INIT_WEAVE = _os.environ.get("K_INIT_WEAVE", "1") == "1"
XP_HW = _os.environ.get("K_XP_HW", "1") == "1"

_PROG_CACHE = {}


def _slices(SF):
    """512-wide slices, except a trailing 640 remainder splits 384+256 so
    every f32r matmul free dim is >= 256 (below 256 f32r runs at 1/4 rate)."""
    out = []
    lo = 0
    while lo < SF:
        rem = SF - lo
        if rem > 512:
            step = 512 if rem >= 768 else 384
        else:
            step = rem
        out.append((lo, lo + step))
        lo += step
    return out


def _build_program(SKP, reps=1, parts="pako"):
    key = (SKP, reps, parts)
    if key in _PROG_CACHE:
        return _PROG_CACHE[key]

    import concourse.bass as bass
    import concourse.tile as tile
    from concourse import bacc, mybir
    from contextlib import ExitStack

    f32 = mybir.dt.float32
    f32r = mybir.dt.float32r
    bf16 = mybir.dt.bfloat16
    NKC = SKP // P
    ts = bass.ts

    nc = bacc.Bacc("TRN2", target_bir_lowering=False, debug=False, num_devices=NCORES)

    def din(name, shape, dt):
        return nc.dram_tensor(name, list(shape), dt, kind="ExternalInput").ap()

    a = {}
    a["x1T"] = din("x1T", (E, S), bf16)
    a["x2T"] = din("x2T", (E, S), bf16)
    a["x1p"] = din("x1p", (E, SKP), bf16)
    a["x2p"] = din("x2p", (E, SKP), bf16)
    for n in ("q1", "q2", "k1", "k2", "v1", "v2"):
        a["W" + n] = din("W" + n, (E, EG), bf16)
    a["Wo1"] = din("Wo1", (EG, S), bf16)
    a["Wo2"] = din("Wo2", (EG, S), bf16)
    for n in ("q1", "q2", "k1", "k2"):
        a["b" + n] = din("b" + n, (P, 4), f32)
    a["bv1"] = din("bv1", (P, EG), f32)
    a["bv2"] = din("bv2", (P, EG), f32)
    for n in ("cq1", "sq1", "cq2", "sq2"):
        a[n] = din(n, (P, S), bf16)
    for n in ("ck1", "sk1", "ck2", "sk2"):
        a[n] = din(n, (P, SKP), bf16)
    a["mb1"] = din("mb1", (P, NKC), f32)
    a["mb2"] = din("mb2", (P, NKC), f32)
    a["perm"] = din("perm", (P, P), bf16)
    if USE_SELMAT:
        a["selmat"] = din("selmat", (P, P), f32)
        a["zeros"] = din("zeros", (P, 1024), f32)
    out1 = nc.dram_tensor("o1", [S, E], bf16, kind="ExternalOutput").ap()
    out2 = nc.dram_tensor("o2", [S, E], bf16, kind="ExternalOutput").ap()

    Exp = mybir.ActivationFunctionType.Exp

    def emit(tc):
        with ExitStack() as ctx:
            consts = ctx.enter_context(tc.tile_pool(name="consts", bufs=1))
            xpool = ctx.enter_context(tc.tile_pool(name="xpool", bufs=1))
            xpp = ctx.enter_context(tc.tile_pool(name="xpp", bufs=1))
            wpool = ctx.enter_context(tc.tile_pool(name="wpool", bufs=6))
            qkv = ctx.enter_context(tc.tile_pool(name="qkv", bufs=1))
            attp = ctx.enter_context(tc.tile_pool(name="attp", bufs=1))
            upool = ctx.enter_context(tc.tile_pool(name="upool", bufs=2))
            sm = ctx.enter_context(tc.tile_pool(name="sm", bufs=2))
            # PSUM: ST 2x[P,1024](4 banks) + proj 2x[P,512](2) + o 2x[P,512](2)
            stps = ctx.enter_context(tc.tile_pool(name="stps", bufs=2, space="PSUM"))
            pjps = ctx.enter_context(tc.tile_pool(name="pjps", bufs=2, space="PSUM"))
            ops_ = ctx.enter_context(tc.tile_pool(name="ops", bufs=2, space="PSUM"))

            dmas = [nc.sync, nc.scalar]
            dma_ctr = [0]

            def next_dma():
                e = dmas[dma_ctr[0] % 2]
                dma_ctr[0] += 1
                return e

            def cload(name, shape, dt):
                t = consts.tile(list(shape), dt, tag=name)
                nc.gpsimd.dma_start(t[:], a[name])
                return t

            perm_t = cload("perm", (P, P), bf16)
            mb = {1: cload("mb1", (P, NKC), f32), 2: cload("mb2", (P, NKC), f32)}
            cq = {n: cload(n, (P, S), bf16) for n in ("cq1", "sq1", "cq2", "sq2")}
            ck = {n: cload(n, (P, SKP), bf16) for n in ("ck1", "sk1", "ck2", "sk2")}
            bqk = {n: cload("b" + n, (P, 4), f32) for n in ("q1", "q2", "k1", "k2")}
            bv = {n: cload("b" + n, (P, EG), f32) for n in ("v1", "v2")}
            if USE_SELMAT:
                selmat = cload("selmat", (P, P), f32)
                zrc = [consts.tile([P, 1024], f32, tag=f"zrc{i}", name=f"zrc{i}")
                       for i in range(2)]
                for i in range(2):
                    nc.gpsimd.dma_start(zrc[i][:], a["zeros"])

            def body(_=None):
                # ---------- generators (work-unit granularity) ----------
                def load_x(pool, slot_tag, ap, SF, split):
                    t = pool.tile([P, KE, SF], bf16, tag=slot_tag, name="xt")
                    src = ap.rearrange("(ko p) s -> p ko s", p=P)
                    if XP_HW:
                        next_dma().dma_start(t[:, :, 0:split], src[:, :, 0:split])
                        next_dma().dma_start(t[:, :, split:SF], src[:, :, split:SF])
                    else:
                        nc.gpsimd.dma_start(t[:], src)
                    return t

                def load_w_halves(wname, dt_, SFW):
                    ko = a[wname].shape[0] // P // 2  # k-chunks per half
                    tiles = []
                    for h in range(2):
                        w_t = wpool.tile([P, ko, SFW], dt_, tag="w", name="wt")
                        next_dma().dma_start(
                            w_t[:],
                            a[wname].rearrange("(h ko p) m -> h p ko m", h=2, p=P)[h],
                        )
                        tiles.append(w_t)
                    return tiles

                def proj_qk(dst, x_t, wname, SF, cos_t, sin_t, w_ts=None):
                    """Generator: dst [P, 4, SF] bf16; yields per (mi, nb)."""
                    if w_ts is None:
                        w_ts = load_w_halves("W" + wname, bf16, EG)
                    slices = _slices(SF)
                    for mi in range(4):
                        for lo, hi in slices:
                            ps = pjps.tile([P, 512], f32, tag="pj", name="pj")
                            for ki in range(KE):
                                nc.tensor.matmul(
                                    ps[:, : hi - lo],
                                    w_ts[ki // 4][:, ki % 4, ts(mi, P)],
                                    x_t[:, ki, lo:hi],
                                    start=(ki == 0),
                                    stop=(ki == KE - 1),
                                )
                            nc.vector.tensor_scalar_add(
                                dst[:, mi, lo:hi],
                                ps[:, : hi - lo],
                                bqk[wname][:, mi : mi + 1],
                            )
                            # rope (every slab has rot rows 0-31 / 64-95)
                            pp = pjps.tile([P, 512], f32, tag="pj", name="pp")
                            nc.tensor.matmul(
                                pp[:, : hi - lo], perm_t[:], dst[:, mi, lo:hi],
                                start=True, stop=True,
                            )
                            tmp = sm.tile([P, 512], bf16, tag="tmp", name="tmp")
                            nc.vector.tensor_mul(
                                tmp[:, : hi - lo], pp[:, : hi - lo],
                                sin_t[:, lo:hi],
                            )
                            nc.vector.tensor_mul(
                                dst[:, mi, lo:hi], dst[:, mi, lo:hi],
                                cos_t[:, lo:hi],
                            )
                            nc.vector.tensor_add(
                                dst[:, mi, lo:hi], dst[:, mi, lo:hi],
                                tmp[:, : hi - lo],
                            )
                            yield

                def proj_v(dst, xp_t, wname):
                    """Generator: dst [P, NKC, 4*VPAIR] bf16; yields per mi."""
                    w_ts = load_w_halves("W" + wname, bf16, EG)
                    dstv = dst[:].rearrange("p k (pr c) -> p k pr c", c=VPAIR)
                    psv = lambda t: t.rearrange("p (pr h c) -> p pr h c", pr=4, h=2)
                    bvv = psv(bv[wname][:])
                    for mi in range(NKC):
                        ps = pjps.tile([P, 512], f32, tag="pj", name="pjv")
                        for ki in range(KE):
                            nc.tensor.matmul(
                                ps[:],
                                xp_t[:, ki, ts(mi, P)],
                                w_ts[ki // 4][:, ki % 4, :],
                                start=(ki == 0),
                                stop=(ki == KE - 1),
                            )
                        pp = psv(ps[:])
                        nc.vector.tensor_add(
                            dstv[:, mi, :, 0:D], pp[:, :, 0, :], bvv[:, :, 0, :]
                        )
                        nc.vector.tensor_add(
                            dstv[:, mi, :, 129 : 129 + D],
                            pp[:, :, 1, :], bvv[:, :, 1, :],
                        )
                        yield

                def attention(att_t, first, qT, kT, v_t, mbias):
                    """Generator: yields per (sq, gp, ki)x2 and per normalize.
                    All NKC score/exp units are emitted before the AV chain so
                    the o-psum slot wait (previous pair's normalize) hides
                    under score work on the in-order PE stream."""
                    do_k = "k" in parts
                    ut_bufs = (NKC + 1) if ATT_SPLIT else 2
                    for sq in range(2):
                        for gp in range(4):  # slab = heads (2gp, 2gp+1)
                            hA, hB = 2 * gp, 2 * gp + 1
                            uts = []
                            if not ATT_SPLIT and do_k:
                                oA = ops_.tile([P, 512], f32, tag="o", name="oA")
                                oB = ops_.tile([P, 512], f32, tag="o", name="oB")
                            for ki in range(NKC):
                                st = stps.tile([P, 1024], f32, tag="st", name="st")
                                for half, par in ((0, 0), (1, 1)):
                                    nc.tensor.matmul(
                                        st[:, 512 * par : 512 * par + 512],
                                        kT[64 * half : 64 * half + 64, gp, ts(ki, P)],
                                        qT[64 * half : 64 * half + 64, gp, ts(sq, 512)],
                                        start=True,
                                        stop=True,
                                    )
                                ut = upool.tile([P, 1024], bf16, tag="ut",
                                                name="ut", bufs=ut_bufs)
                                nc.scalar.activation(
                                    ut[:], st[:], Exp,
                                    bias=mbias[:, ki : ki + 1], scale=SCALE,
                                )
                                uts.append(ut)
                                if not ATT_SPLIT and do_k:
                                    for ki2 in (ki,):
                                        nc.tensor.matmul(
                                            oA[0:65, :],
                                            v_t[:, ki2, VPAIR * gp : VPAIR * gp + 65],
                                            ut[:, 0:512],
                                            start=(ki2 == 0),
                                            stop=(ki2 == NKC - 1),
                                        )
                                        nc.tensor.matmul(
                                            oB[:, :],
                                            v_t[:, ki2, VPAIR * gp + 65 : VPAIR * gp + 193],
                                            ut[:, 512:1024],
                                            start=(ki2 == 0),
                                            stop=(ki2 == NKC - 1),
                                        )
                                yield
                            if ATT_SPLIT and do_k:
                                oA = ops_.tile([P, 512], f32, tag="o", name="oA")
                                oB = ops_.tile([P, 512], f32, tag="o", name="oB")
                                for ki in range(NKC):
                                    nc.tensor.matmul(
                                        oA[0:65, :],
                                        v_t[:, ki, VPAIR * gp : VPAIR * gp + 65],
                                        uts[ki][:, 0:512],
                                        start=(ki == 0),
                                        stop=(ki == NKC - 1),
                                    )
                                    nc.tensor.matmul(
                                        oB[:, :],
                                        v_t[:, ki, VPAIR * gp + 65 : VPAIR * gp + 193],
                                        uts[ki][:, 512:1024],
                                        start=(ki == 0),
                                        stop=(ki == NKC - 1),
                                    )
                                    yield
                            if not do_k:
                                continue
                            # normalize: full-tile 1/r (single-row custom-DVE
                            # recip is broken); r_A sits at oA row 64, r_B at
                            # oB row 63. One DMA stages recip rows 63-64 to
                            # partitions 0-1, partition_broadcast replicates,
                            # then partition-aligned multiplies.
                            rc = sm.tile([P, 1024], f32, tag="rc", name="rc")
                            nc.vector.reciprocal_approx_fast(
                                rc[:, 0:512], oA[:]
                            )
                            nc.vector.reciprocal_approx_fast(
                                rc[:, 512:1024], oB[:]
                            )
                            stage = sm.tile([P, 1024], f32, tag="stg",
                                            name="stg")
                            nc.gpsimd.dma_start(
                                stage[0:1, 0:512], rc[64:65, 0:512]
                            )
                            nc.gpsimd.dma_start(
                                stage[0:1, 512:1024], rc[63:64, 512:1024]
                            )
                            # pbcast writes must start at partition 0; head B
                            # needs its factor at partitions 64-127, so its
                            # broadcast covers all 128 (separate columns)
                            rrs = sm.tile([P, 1024], f32, tag="rrs",
                                          name="rrs")
                            nc.gpsimd.partition_broadcast(
                                rrs[0:128, :], stage[0:1, :]
                            )
                            dst = att_t[:, gp, ts(sq, 512)]
                            if first:
                                nc.vector.tensor_mul(
                                    att_t[0:64, gp, ts(sq, 512)],
                                    oA[0:64, :], rrs[0:64, 0:512],
                                )
                                nc.vector.tensor_mul(
                                    att_t[64:128, gp, ts(sq, 512)],
                                    oB[64:128, :], rrs[64:128, 512:1024],
                                )
                            else:
                                tmpo = sm.tile([P, 512], bf16, tag="tmpo", name="tm")
                                nc.vector.tensor_mul(
                                    tmpo[0:64, :], oA[0:64, :], rrs[0:64, 0:512]
                                )
                                nc.vector.tensor_mul(
                                    tmpo[64:128, :], oB[64:128, :],
                                    rrs[64:128, 512:1024],
                                )
                                nc.vector.tensor_add(dst, dst, tmpo[:])
                            yield

                def outproj(att_t, wo_name, out_ap, wo_ts=None):
                    """Generator: yields per si."""
                    if wo_ts is None:
                        wo_ts = load_w_halves(wo_name, bf16, S)
                    for si in range(8):
                        osb = sm.tile([P, 1024], bf16, tag="ob", name="ob")
                        for fb in range(2):
                            ps = pjps.tile([P, 512], f32, tag="pj", name="pjo")
                            for ei in range(4):
                                nc.tensor.matmul(
                                    ps[:],
                                    att_t[:, ei, ts(si, P)],
                                    wo_ts[ei // 2][:, ei % 2, 512 * fb : 512 * fb + 512],
                                    start=(ei == 0),
                                    stop=(ei == 3),
                                )
                            nc.vector.tensor_copy(
                                osb[:, 512 * fb : 512 * fb + 512], ps[:]
                            )
                        next_dma().dma_start(out_ap[ts(si, P), :], osb[:])
                        yield

                def drain(*gens):
                    for g in gens:
                        for _ in g:
                            pass

                def drain_n(g, n):
                    for _ in range(n):
                        try:
                            next(g)
                        except StopIteration:
                            return False
                    return True

                def weave(mains, sides=(), ratio=3):
                    """Pull `ratio` units per round from each main gen, then
                    ONE unit from the head of the side chain (sides drain
                    sequentially -- this keeps per-pool slot demand bounded).
                    sides: [(gen, cap), ...]; cap limits units pulled here."""
                    mains = [[g, False] for g in mains]
                    chain = [[g, cap] for g, cap in sides]
                    while True:
                        alive = False
                        for rec in mains:
                            if rec[1]:
                                continue
                            for _ in range(ratio):
                                try:
                                    next(rec[0])
                                    alive = True
                                except StopIteration:
                                    rec[1] = True
                                    break
                        while chain:
                            g, cap = chain[0]
                            if cap <= 0:
                                chain.pop(0)
                                continue
                            try:
                                next(g)
                                chain[0][1] -= 1
                            except StopIteration:
                                chain.pop(0)
                                continue
                            break
                        if not alive:
                            break

                # ---------- tiles ----------
                qT1 = qkv.tile([P, 4, S], bf16, tag="qT1")
                qT2 = qkv.tile([P, 4, S], bf16, tag="qT2")
                kT1 = qkv.tile([P, 4, SKP], bf16, tag="kT1")
                kT2 = qkv.tile([P, 4, SKP], bf16, tag="kT2")
                v1 = qkv.tile([P, NKC, 4 * VPAIR], bf16, tag="v1")
                v2 = qkv.tile([P, NKC, 4 * VPAIR], bf16, tag="v2")
                att1 = attp.tile([P, 4, S], bf16, tag="att1")
                att2 = attp.tile([P, 4, S], bf16, tag="att2")

                # ones / zero columns of augmented V (proj_v writes V slots)
                for v_t in (v1, v2):
                    vv = v_t[:].rearrange("p k (pr c) -> p k pr c", c=VPAIR)
                    nc.vector.memset(vv[:, :, :, D : D + 1], 1.0)
                    nc.vector.memset(vv[:, :, :, 128:129], 1.0)
                    nc.vector.memset(vv[:, :, :, 65:128], 0.0)

                # ---------- schedule ----------
                wq1_ts = load_w_halves("Wq1", bf16, EG)
                x1t = load_x(xpool, "x", a["x1T"], S, 512)
                xp2 = load_x(xpp, "xp", a["x2p"], SKP, 384)
                g_q1 = proj_qk(qT1, x1t, "q1", S, cq["cq1"], cq["sq1"],
                               w_ts=wq1_ts)
                g_k2 = proj_qk(kT2, xp2, "k2", SKP, ck["ck2"], ck["sk2"])
                g_v2 = proj_v(v2, xp2, "v2")
                if INIT_WEAVE:
                    weave([g_q1], sides=[(g_k2, 99), (g_v2, 99)], ratio=1)
                drain(g_q1, g_k2, g_v2)

                do_a = "a" in parts
                xp1 = load_x(xpp, "xp", a["x1p"], SKP, 384)
                x2t = load_x(xpool, "x", a["x2T"], S, 512)
                g_k1 = proj_qk(kT1, xp1, "k1", SKP, ck["ck1"], ck["sk1"])
                g_v1 = proj_v(v1, xp1, "v1")
                g_q2 = proj_qk(qT2, x2t, "q2", S, cq["cq2"], cq["sq2"])
                if do_a:
                    # combo unit counts per sq-half
                    upg = (2 * NKC + 1) if ATT_SPLIT else (NKC + 1)
                    half_units = 4 * upg if "k" in parts else 4 * NKC
                    c0 = attention(att1, True, qT1, kT2, v2, mb[2])
                    weave([c0], sides=[(g_k1, 99), (g_v1, 99), (g_q2, 99)], ratio=3)
                    drain(g_k1, g_v1, g_q2)
                    wo1_ts = load_w_halves("Wo1", bf16, S)
                    wo2_ts = load_w_halves("Wo2", bf16, S)
                    c1 = attention(att1, False, qT1, kT1, v1, mb[1])
                    c2 = attention(att2, True, qT2, kT1, v1, mb[1])
                    c3 = attention(att2, False, qT2, kT2, v2, mb[2])
                    if "o" in parts:
                        op1 = outproj(att1, "Wo1", out1, wo_ts=wo1_ts)
                        op2 = outproj(att2, "Wo2", out2, wo_ts=wo2_ts)
                        # op si<4 needs sq0 of its att complete; si>=4 needs all
                        drain_n(c1, half_units)
                        weave([c1], sides=[(op1, 4)], ratio=3)
                        weave([c2], sides=[(op1, 99)], ratio=3)
                        drain(op1)
                        drain_n(c3, half_units)
                        weave([c3], sides=[(op2, 4)], ratio=3)
                        drain(op2)
                    else:
                        drain(c1, c2, c3)
                        z = sm.tile([P, 1024], f32, tag="ob")
                        nc.vector.memset(z[:], 0.0)
                        nc.sync.dma_start(out1[0:P, :], z[:])
                        nc.sync.dma_start(out2[0:P, :], z[:])
                else:
                    drain(g_k1, g_v1, g_q2)
                    z = sm.tile([P, 1024], f32, tag="ob")
                    nc.vector.memset(z[:], 0.0)
                    nc.sync.dma_start(out1[0:P, :], z[:])
                    nc.sync.dma_start(out2[0:P, :], z[:])

            if reps > 1:
                with tc.For_i(
                    0, reps, 1,
                    hint_engines=(
                        mybir.EngineType.PE,
                        mybir.EngineType.Activation,
                        mybir.EngineType.DVE,
                        mybir.EngineType.SP,
                        mybir.EngineType.Pool,
                    ),
                ):
                    body()
            else:
                body()

    with tile.TileContext(nc) as tc:
        emit(tc)
    nc.compile()
    _PROG_CACHE[key] = nc
    return nc


def _prep_inputs(inputs):
    """Host-side sharding/packing. Returns (in_maps, SKP, bo1, bo2)."""
    f32 = np.float32
    x1 = np.asarray(inputs["x1"], f32)
    x2 = np.asarray(inputs["x2"], f32)
    m1 = np.asarray(inputs["x1_padding_mask"]).astype(np.int64)
    m2 = np.asarray(inputs["x2_padding_mask"]).astype(np.int64)
    cos1 = np.asarray(inputs["cos1"], f32).reshape(S, R)
    sin1 = np.asarray(inputs["sin1"], f32).reshape(S, R)
    cos2 = np.asarray(inputs["cos2"], f32).reshape(S, R)
    sin2 = np.asarray(inputs["sin2"], f32).reshape(S, R)

    idx1 = [np.nonzero(m1[b])[0] for b in range(B)]
    idx2 = [np.nonzero(m2[b])[0] for b in range(B)]
    maxn = max([len(i) for i in idx1] + [len(i) for i in idx2] + [1])
    SKP = ((maxn + P - 1) // P) * P
    NKC = SKP // P

    import ml_dtypes

    bf16 = ml_dtypes.bfloat16

    gw = []
    for g in range(2):
        rows = slice(g * EG, (g + 1) * EG)
        d = {}
        for n in ("q1", "q2", "k1", "k2", "v1", "v2"):
            W = np.asarray(inputs["W" + n], f32)[rows]
            bb = np.asarray(inputs["b" + n], f32)[rows]
            d["W" + n] = np.ascontiguousarray(W.T).astype(bf16)
            if n[0] == "v":
                d["b" + n] = np.ascontiguousarray(
                    np.broadcast_to(bb[None, :], (P, EG))
                )
            else:
                d["b" + n] = np.ascontiguousarray(bb.reshape(4, P).T)
        for n in ("o1", "o2"):
            W = np.asarray(inputs["W" + n], f32)[:, rows]
            d["W" + n] = np.ascontiguousarray(W.T).astype(bf16)
        gw.append(d)

    def rope_rows(tab, fill):
        """[128, S-like] rows: per 64-block [32 rope rows | 32 fill rows],
        repeated for the 2 heads of a slab (all slabs identical)."""
        n = tab.shape[1]
        blk = np.concatenate(
            [tab, np.full((D - R, n), fill, f32)], axis=0
        )  # [64, n]
        return np.concatenate([blk, blk], axis=0)  # [128, n]

    cq = {
        "cq1": rope_rows(cos1.T, 1.0).astype(bf16),
        "sq1": rope_rows(sin1.T, 0.0).astype(bf16),
        "cq2": rope_rows(cos2.T, 1.0).astype(bf16),
        "sq2": rope_rows(sin2.T, 0.0).astype(bf16),
    }
    # rotate_half perm: rot rows at 0-31 / 64-95 of each slab; pass rows
    # (32-63 / 96-127) have all-zero columns.
    pm = np.zeros((P, P), f32)
    for o in (0, 64):
        for i in range(16):
            pm[o + 16 + i, o + i] = -1.0
            pm[o + i, o + 16 + i] = 1.0

    in_maps = []
    for c in range(NCORES):
        b, g = c // 2, c % 2
        m = {}
        m["x1T"] = np.ascontiguousarray(x1[b].T).astype(bf16)
        m["x2T"] = np.ascontiguousarray(x2[b].T).astype(bf16)
        for which, xb, idx, cos, sin in (
            (1, x1[b], idx1[b], cos1, sin1),
            (2, x2[b], idx2[b], cos2, sin2),
        ):
            n = len(idx)
            xp = np.zeros((SKP, E), f32)
            xp[:n] = xb[idx]
            m[f"x{which}p"] = np.ascontiguousarray(xp.T).astype(bf16)
            ckk = np.zeros((R, SKP), f32)
            skk = np.zeros((R, SKP), f32)
            ckk[:, :n] = cos.T[:, idx]
            skk[:, :n] = sin.T[:, idx]
            m[f"ck{which}"] = rope_rows(ckk, 1.0).astype(bf16)
            m[f"sk{which}"] = rope_rows(skk, 0.0).astype(bf16)
            mbv = np.full(SKP, MASK_BIAS, f32)
            mbv[:n] = 0.0
            m[f"mb{which}"] = np.ascontiguousarray(mbv.reshape(NKC, P).T)
        for n in ("q1", "q2", "k1", "k2", "v1", "v2"):
            m["W" + n] = gw[g]["W" + n]
            m["b" + n] = gw[g]["b" + n]
        m["Wo1"] = gw[g]["Wo1"]
        m["Wo2"] = gw[g]["Wo2"]
        m.update(cq)
        m["perm"] = pm.astype(bf16)
        if USE_SELMAT:
            sel = np.zeros((P, P), f32)
            sel[0, :64] = 1.0
            m["selmat"] = sel
            m["zeros"] = np.zeros((P, 1024), f32)
        in_maps.append(m)

    bo1 = np.asarray(inputs["bo1"], f32)
    bo2 = np.asarray(inputs["bo2"], f32)
    return in_maps, SKP, bo1, bo2


def kernel(**inputs):
    from concourse.bass_utils import run_bass_kernel_spmd

    in_maps, SKP, bo1, bo2 = _prep_inputs(inputs)
    nc = _build_program(SKP)
    res = run_bass_kernel_spmd(nc, in_maps, core_ids=list(range(NCORES)))
    f32 = np.float32
    o1 = np.stack(
        [res.results[2 * b]["o1"].astype(f32)
         + res.results[2 * b + 1]["o1"].astype(f32) + bo1 for b in range(B)]
    )
    o2 = np.stack(
        [res.results[2 * b]["o2"].astype(f32)
         + res.results[2 * b + 1]["o2"].astype(f32) + bo2 for b in range(B)]
    )
    return o1.astype(np.float32), o2.astype(np.float32)
